# revision 19
# baseline (speedup 1.0000x reference)
"""Trainium2 Bass kernel for nn_ChannelSELayerOwn (topk channel masking).

Reference computation (per batch sample b of 8, data-parallel across 8 cores):
  y   = mean(x[b], axis=(D,H,W))                       # (64,)
  h   = leaky_relu(w1 @ y + b1, 0.01)                  # (64,)
  z   = w2 @ h + b2                                    # (64,) pre-sigmoid logits
  idx = top_8 indices of sigmoid(z) == top_8 of z      # sigmoid is monotonic
  out[b] = x[b, idx]                                   # (8, D, H, W), bit-exact copy

Device kernel per core (one sample):
  phase A: stream x (viewed as 128 x 55296) through SBUF, reduce_sum per tile
  phase B: pair-reduce partitions via a tiny matmul (R matrix also folds in
           the 1/DHW mean divisor), fp32 FC stack on PE, top-8 via the DVE
           max8/max-index instructions on the pre-sigmoid logits
  phase C: indirect-DMA gather of the 8 selected channels, with each channel
           split 16 ways across partitions (full 128-partition DMA width),
           then one contiguous store to the output
"""

import os

import numpy as np

import concourse.bacc as bacc
import concourse.bass as bass
import concourse.mybir as mybir
from concourse import tile
from concourse.bass_utils import run_bass_kernel_spmd

F32 = mybir.dt.float32
U32 = mybir.dt.uint32

B, C, D, H, W = 8, 64, 48, 48, 48
M = D * H * W              # 110592 elements per channel
R_TOP = 8                  # channels kept
NEG_SLOPE = 0.01
N_CORES = 8

SPLIT = 16                 # each gathered channel spread over 16 partitions
MSUB = M // SPLIT          # 6912 elements per partition in the gather
TF = 6912                  # streaming tile free-dim (55296 = 8 * 6912)
NT = (M * C // 128) // TF  # 8 streaming tiles
GCHUNK = 8                 # gather pipeline chunks
GF = MSUB // GCHUNK        # 864 elements per gather chunk
TAIL_SPLIT = 4             # the last streaming tile is split this many ways
TFS = TF // TAIL_SPLIT     # 1728-wide sub-tiles for a short reduce tail

# results of the most recent run_bass_kernel_spmd call (for test harness use)
LAST_RESULTS = None


def build_nc():
    nc = bacc.Bacc("TRN2", target_bir_lowering=False)

    x_d = nc.dram_tensor("x", [C, M], F32, kind="ExternalInput")
    w1r_d = nc.dram_tensor("w1r", [128, C], F32, kind="ExternalInput")
    b1c_d = nc.dram_tensor("b1c", [C, 1], F32, kind="ExternalInput")
    w2t_d = nc.dram_tensor("w2t", [C, C], F32, kind="ExternalInput")
    b2r_d = nc.dram_tensor("b2r", [1, C], F32, kind="ExternalInput")
    ones_d = nc.dram_tensor("ones1", [1, 128], F32, kind="ExternalInput")
    sel_d = nc.dram_tensor("sel16", [128, R_TOP], F32, kind="ExternalInput")
    mod_d = nc.dram_tensor("mod16", [128, 1], F32, kind="ExternalInput")
    out_d = nc.dram_tensor("out", [R_TOP, M], F32, kind="ExternalOutput")

    # x as 128 partitions x 55296: partition 2c+t holds half t of channel c
    x_stream = x_d[:].rearrange("c (t m) -> (c t) m", t=2)
    # x as a 1024 x 6912 table for the gather: row 16c+s = sixteenth s of channel c
    x_table = x_d[:].rearrange("c (s m) -> (c s) m", s=SPLIT)
    out_wr = out_d[:].rearrange("r (s m) -> (r s) m", s=SPLIT)

    with tile.TileContext(nc) as tc:
        with (
            tc.tile_pool(name="consts", bufs=1) as cpool,
            tc.tile_pool(name="stream", bufs=4) as spool,
            tc.tile_pool(name="small", bufs=1) as mpool,
            tc.tile_pool(name="gather", bufs=4) as gpool,
            tc.tile_pool(name="psum", bufs=1, space="PSUM") as ppool,
        ):
            w1r = cpool.tile([128, C], F32)
            nc.scalar.dma_start(w1r[:], w1r_d[:])
            w2t = cpool.tile([C, C], F32)
            nc.scalar.dma_start(w2t[:], w2t_d[:])
            b1c = cpool.tile([C, 1], F32)
            nc.scalar.dma_start(b1c[:], b1c_d[:])
            b2r = cpool.tile([1, C], F32)
            nc.scalar.dma_start(b2r[:], b2r_d[:])
            ones1 = cpool.tile([1, 128], F32)
            nc.scalar.dma_start(ones1[:], ones_d[:])
            sel16 = cpool.tile([128, R_TOP], F32)
            nc.scalar.dma_start(sel16[:], sel_d[:])
            mod16 = cpool.tile([128, 1], F32)
            nc.scalar.dma_start(mod16[:], mod_d[:])

            # ---- phase A: streaming channel sums ----
            ctxA = nc.named_scope("phaseA"); ctxA.__enter__()
            # all loads on one HWDGE ring: FIFO, so tile j completes before
            # tile j+1 and the reducer starts early.  The final full tile is
            # split into small sub-tiles so the last reduce adds ~2us, not
            # a full 7us, after the last load lands.
            npart = (NT - 1) + TAIL_SPLIT
            partials = mpool.tile([128, npart], F32)
            col = 0
            for j in range(NT - 1):
                xt = spool.tile([128, TF], F32, tag="xt")
                # the gpsimd SWDGE sequencer boots ~6us before the HWDGE
                # rings; issuing the first load there starts the HBM stream
                # (and the in-order reducer) earlier
                eng = nc.gpsimd if j == 0 else nc.sync
                eng.dma_start(xt[:], x_stream[:, j * TF : (j + 1) * TF])
                nc.vector.reduce_sum(
                    partials[:, col : col + 1], xt[:], axis=mybir.AxisListType.X
                )
                col += 1
            base = (NT - 1) * TF
            for j in range(TAIL_SPLIT):
                xts = spool.tile([128, TFS], F32, tag="xts")
                nc.sync.dma_start(
                    xts[:], x_stream[:, base + j * TFS : base + (j + 1) * TFS]
                )
                nc.vector.reduce_sum(
                    partials[:, col : col + 1], xts[:], axis=mybir.AxisListType.X
                )
                col += 1
            total = mpool.tile([128, 1], F32)
            nc.vector.reduce_sum(total[:], partials[:], axis=mybir.AxisListType.X)

            ctxA.__exit__(None, None, None)
            # ---- phase B: means -> FC1 -> leaky -> FC2 -> top-8 ----
            ctxB = nc.named_scope("phaseB"); ctxB.__enter__()
            # h = leaky(w1 @ mean + b1): the pair-reduce and 1/M divisor are
            # folded into w1r (host-side W1R = R @ w1.T), so FC1 consumes the
            # raw 128-partition sums directly
            h_ps = ppool.tile([C, 1], F32)
            nc.tensor.matmul(h_ps[:], lhsT=w1r[:], rhs=total[:], start=True, stop=True)
            h = mpool.tile([C, 1], F32)
            nc.vector.tensor_add(h[:], h_ps[:], b1c[:])
            h_scaled = mpool.tile([C, 1], F32)
            nc.vector.tensor_scalar_mul(h_scaled[:], h[:], NEG_SLOPE)
            h_act = mpool.tile([C, 1], F32)
            nc.vector.tensor_tensor(h_act[:], h[:], h_scaled[:], op=mybir.AluOpType.max)

            # z in row layout directly: z_row = h.T @ w2.T  (lhsT=h, rhs=w2t)
            zrow_ps = ppool.tile([1, C], F32)
            nc.tensor.matmul(zrow_ps[:], lhsT=h_act[:], rhs=w2t[:], start=True, stop=True)
            zrow = mpool.tile([1, C], F32)
            nc.vector.tensor_add(zrow[:], zrow_ps[:], b2r[:])

            m8 = mpool.tile([1, R_TOP], F32)
            nc.vector.max(m8[:], zrow[:])
            idx8 = mpool.tile([1, R_TOP], U32)
            nc.vector.max_index(idx8[:], m8[:], zrow[:])

            # ---- offsets: one index per dest partition p (HW contract):
            # offs[p] = idx[p//16]*16 + p%16.  idx is broadcast to all 128
            # partitions with a K=1 outer-product matmul, then a one-hot
            # selector (premultiplied by 16) picks column p//16. ----
            idx8f = mpool.tile([1, R_TOP], F32)
            nc.vector.tensor_copy(idx8f[:], idx8[:])
            bc_ps = ppool.tile([128, R_TOP], F32)
            nc.tensor.matmul(bc_ps[:], lhsT=ones1[:], rhs=idx8f[:], start=True, stop=True)
            selprod = mpool.tile([128, R_TOP], F32)
            nc.vector.tensor_mul(selprod[:], bc_ps[:], sel16[:])
            offsf = mpool.tile([128, 1], F32)
            nc.vector.reduce_sum(offsf[:], selprod[:], axis=mybir.AxisListType.X)
            nc.vector.tensor_add(offsf[:], offsf[:], mod16[:])
            offs = mpool.tile([128, 1], U32)
            nc.vector.tensor_copy(offs[:], offsf[:])

            ctxB.__exit__(None, None, None)
            # ---- phase C: gather selected channels at full partition width ----
            ctxC = nc.named_scope("phaseC"); ctxC.__enter__()
            for g in range(GCHUNK):
                gt = gpool.tile([128, GF], F32, tag="gt")
                nc.gpsimd.indirect_dma_start(
                    out=gt[:],
                    out_offset=None,
                    in_=x_table,
                    in_offset=bass.IndirectOffsetOnAxis(ap=offs[:], axis=0),
                    element_offset=g * GF,
                )
                eng = nc.sync if g % 2 == 0 else nc.scalar
                eng.dma_start(out_wr[:, g * GF : (g + 1) * GF], gt[:])

            ctxC.__exit__(None, None, None)

    nc.compile()
    return nc


def _aux_inputs(w1, b1, w2, b2):
    # R[p, p//2] = 1/M so that R.T @ partition_sums = per-channel means
    rmat = np.zeros((128, C), dtype=np.float32)
    rmat[np.arange(128), np.arange(128) // 2] = np.float32(1.0 / M)
    p_arr = np.arange(128)
    return {
        "w1r": np.ascontiguousarray(rmat @ w1.T, dtype=np.float32),
        "b1c": np.ascontiguousarray(b1.reshape(C, 1), dtype=np.float32),
        "w2t": np.ascontiguousarray(w2.T, dtype=np.float32),
        "b2r": np.ascontiguousarray(b2.reshape(1, C), dtype=np.float32),
        "ones1": np.ones((1, 128), dtype=np.float32),
        "sel16": (
            float(SPLIT)
            * (p_arr[:, None] // SPLIT == np.arange(R_TOP)[None, :])
        ).astype(np.float32),
        "mod16": (p_arr % SPLIT).astype(np.float32).reshape(128, 1),
    }


def kernel(x, w1, b1, w2, b2):
    global LAST_RESULTS
    x = np.asarray(x, dtype=np.float32)
    aux = _aux_inputs(
        np.asarray(w1, np.float32), np.asarray(b1, np.float32),
        np.asarray(w2, np.float32), np.asarray(b2, np.float32),
    )
    nc = build_nc()
    in_maps = [
        {"x": np.ascontiguousarray(x[b].reshape(C, M)), **aux} for b in range(B)
    ]
    res = run_bass_kernel_spmd(
        nc,
        in_maps,
        core_ids=list(range(N_CORES)),
        trace=bool(int(os.environ.get("BASS_PROFILE", "0"))),
    )
    LAST_RESULTS = res
    out = np.stack([res.results[b]["out"] for b in range(B)], axis=0)
    return out.reshape(B, R_TOP, D, H, W)
